# revision 6
# baseline (speedup 1.0000x reference)
"""Neighbor aggregation (gnn message passing) Bass kernel for Trainium2.

out[b, i] = sum_{e: src[e]==i} w[e] * H[b, dst[e]]   (per batch b)

8 NeuronCores: core = 2*b + s handles batch b, src-half s (output rows
[s*25000, (s+1)*25000)).

v2 architecture (replaces the SWDGE dma_gather per-edge path, which was
Pool-engine descriptor-generation bound at ~9 ns/token):

  - Gather is done on the Tensor engine: edges are grouped host-side by
    128-row dst-block; per block a one-hot matrix OnehotT [128 dst x 1024
    edge-slots] (bf16, with w folded in) is streamed from HBM and used as
    the stationary matmul operand against the H block [128 x 64] (bf16,
    resident in SBUF), producing messages w*H[dst] token-major in PSUM.
  - Scatter stays on SWDGE dma_scatter_add (CCE f32, parity-split SBUF
    accumulators, one 1024-token call per dst-block; pad tails use the
    junk row PAD_ROW).
  - RMW-hazard safety: each per-block call only contains round-0 edges
    (first occurrence of each src within the block) -> all srcs within a
    call are distinct.  Remaining edges (~2%: in-block src duplicates and
    over-cap overflow) go through a small baseline-style SWDGE
    gather+scatter pass with round-packing.

Hardware constraints (probed previously):
  - SWDGE calls limited to 1024-1152 tokens (descriptor ring packets).
  - dma_scatter_add loses RMW updates when the same dst row repeats in
    close proximity within a call; hence distinct-src calls + PAD_ROW junk
    row (>= 25000) for padding tokens.
"""

import os
import sys

sys.path.insert(0, "/opt/trn_rl_repo")

import numpy as np
import ml_dtypes

import concourse.bacc as bacc
import concourse.mybir as mybir
import concourse.tile as tile
from concourse.bass_utils import run_bass_kernel_spmd

BF16 = ml_dtypes.bfloat16

B, N, E, HS = 4, 50000, 800000, 64
NHALF = N // 2                  # 25000
C = 1024                        # tokens per mini-pass chunk
NGRP = 98                       # accumulator covers idx < 25088
PAD_ROW = 25080                 # junk accumulator row for padding tokens

NBLK = 392                      # dst blocks of 128 rows (covers 50176)
G1 = 8                          # message groups per block
CAP1 = G1 * 128                 # 1024 edge slots per block
QD = NBLK // 4                  # onehot DMA batches (4 blocks each)

LAST_RESULT = {}


def build(nc, ch_mini):
    f32 = mybir.dt.float32
    bf16 = mybir.dt.bfloat16
    i16 = mybir.dt.int16
    i32 = mybir.dt.int32

    h_d = nc.dram_tensor("h", [N, HS], f32, kind="ExternalInput")
    hb_d = nc.dram_tensor("hb", [128, NBLK, HS], bf16, kind="ExternalInput")
    oh_d = nc.dram_tensor("oh", [QD, 128, 4, CAP1], bf16, kind="ExternalInput")
    sx_d = nc.dram_tensor(
        "sx", [NBLK // 8, 128, 8, CAP1 // 16], i16, kind="ExternalInput"
    )
    gidx_d = nc.dram_tensor(
        "gidx", [2, ch_mini, 128, C // 16], i16, kind="ExternalInput"
    )
    sidx_d = nc.dram_tensor(
        "sidx", [2, ch_mini, 128, C // 16], i16, kind="ExternalInput"
    )
    wl_d = nc.dram_tensor(
        "wl", [2, ch_mini, 128, C // 128], f32, kind="ExternalInput"
    )
    acc_d = nc.dram_tensor("acc", [2, 2, 128, NGRP, HS], f32, kind="ExternalOutput")

    with tile.TileContext(nc) as tc:
        with tc.tile_pool(name="accp", bufs=1) as accp, \
             tc.tile_pool(name="hp", bufs=1) as hp, \
             tc.tile_pool(name="ohp", bufs=3) as ohp, \
             tc.tile_pool(name="sxp", bufs=2) as sxp, \
             tc.tile_pool(name="msgp", bufs=6) as msgp, \
             tc.tile_pool(name="pp", bufs=6, space="PSUM") as pp, \
             tc.tile_pool(name="mp", bufs=1) as mp, \
             tc.tile_pool(name="wp", bufs=3) as wp:
            accs = []
            for pr in range(2):
                a0 = accp.tile([128, NGRP, HS], f32, tag=f"acc{pr}0")
                a1 = accp.tile([128, NGRP, HS], f32, tag=f"acc{pr}1")
                nc.vector.memset(a0[:], 0.0)
                nc.vector.memset(a1[:], 0.0)
                accs.append((a0, a1))

            hb_t = hp.tile([128, NBLK, HS], bf16, tag="hb")
            nc.sync.dma_start(hb_t[:], hb_d[:])

            sx_t = None
            for q in range(QD):
                oh_t = ohp.tile([128, 4, CAP1], bf16, tag="oh")
                nc.sync.dma_start(oh_t[:], oh_d[q])
                if q % 2 == 0:
                    sx_t = sxp.tile([128, 8, CAP1 // 16], i16, tag="sx")
                    nc.sync.dma_start(sx_t[:], sx_d[q // 2])
                for bq in range(4):
                    j = q * 4 + bq
                    ps = pp.tile([128, G1, HS], f32, tag="ps")
                    for g in range(G1):
                        nc.tensor.matmul(
                            ps[:, g, :],
                            lhsT=oh_t[:, bq, g * 128:(g + 1) * 128],
                            rhs=hb_t[:, j, :],
                            start=True,
                            stop=True,
                        )
                    msg_t = msgp.tile([128, G1, HS], f32, tag="msg")
                    nc.vector.tensor_copy(msg_t[:], ps[:])
                    a0, a1 = accs[j % 2]
                    nc.gpsimd.dma_scatter_add(
                        out_ap=a0[:],
                        in_ap=msg_t[:],
                        idxs_ap=sx_t[:, bq + 4 * (q % 2), :],
                        num_idxs=CAP1,
                        num_idxs_reg=CAP1,
                        elem_size=HS,
                        sbuf_tokens_per_rank=128,
                        parity_reg=0,
                        out_ap_other=a1[:],
                    )

            # mini pass: src-duplicate and overflow edges via SWDGE gather
            for phase in range(2):
                h_slice = h_d[:][phase * NHALF:(phase + 1) * NHALF, :]
                for k in range(ch_mini):
                    gi = wp.tile([128, C // 16], i16, tag="gi")
                    si = wp.tile([128, C // 16], i16, tag="si")
                    wt = wp.tile([128, C // 128], f32, tag="wt")
                    nc.sync.dma_start(gi[:], gidx_d[phase, k])
                    nc.sync.dma_start(si[:], sidx_d[phase, k])
                    nc.sync.dma_start(wt[:], wl_d[phase, k])

                    msgs = wp.tile([128, C // 128, HS], f32, tag="msgs")
                    nc.gpsimd.dma_gather(
                        out_ap=msgs[:],
                        in_ap=h_slice,
                        idxs_ap=gi[:],
                        num_idxs=C,
                        num_idxs_reg=C,
                        elem_size=HS,
                    )
                    nc.vector.tensor_tensor(
                        out=msgs[:],
                        in0=msgs[:],
                        in1=wt[:].unsqueeze(2).broadcast_to([128, C // 128, HS]),
                        op=mybir.AluOpType.mult,
                    )
                    a0, a1 = accs[(k + phase * ch_mini) % 2]
                    nc.gpsimd.dma_scatter_add(
                        out_ap=a0[:],
                        in_ap=msgs[:],
                        idxs_ap=si[:],
                        num_idxs=C,
                        num_idxs_reg=C,
                        elem_size=HS,
                        sbuf_tokens_per_rank=128,
                        parity_reg=0,
                        out_ap_other=a1[:],
                    )

            for pr in range(2):
                nc.sync.dma_start(acc_d[pr, 0], accs[pr][0][:])
                nc.sync.dma_start(acc_d[pr, 1], accs[pr][1][:])
    return nc


_COMPILED = {}


def _get_compiled(ch_mini):
    if ch_mini not in _COMPILED:
        nc = bacc.Bacc("TRN2", target_bir_lowering=False, debug=False)
        build(nc, ch_mini)
        nc.compile()
        _COMPILED[ch_mini] = nc
    return _COMPILED[ch_mini]


def _wrap16(idx, ch, c):
    a = idx.reshape(ch, c // 16, 16).transpose(0, 2, 1).astype(np.int16)
    return np.ascontiguousarray(np.tile(a, (1, 8, 1)))


def _round_pack(srcs, dsts, ws, cap):
    """Order tokens so equal src never share a chunk: tokens get a within-src
    rank (round); each round starts at a fresh chunk boundary."""
    order = np.argsort(srcs, kind="stable")
    ss = srcs[order]
    n = ss.shape[0]
    if n == 0:
        return (np.zeros(cap, np.int64), np.full(cap, PAD_ROW, np.int64),
                np.zeros(cap, np.float32), 1)
    first = np.r_[True, ss[1:] != ss[:-1]]
    gstart = np.flatnonzero(first)
    rank = np.arange(n) - np.repeat(gstart, np.diff(np.r_[gstart, n]))
    order2 = np.argsort(rank, kind="stable")
    rank_s = rank[order2]
    tok = order[order2]
    nr = np.bincount(rank_s)
    chunks_per_round = -(-nr // cap)
    starts = np.concatenate([[0], np.cumsum(chunks_per_round[:-1] * cap)])
    total_chunks = int(chunks_per_round.sum())
    pos = starts[rank_s] + (np.arange(n) - np.repeat(
        np.concatenate([[0], np.cumsum(nr[:-1])]), nr))
    cap_total = total_chunks * cap
    g = np.zeros(cap_total, np.int64)
    s = np.full(cap_total, PAD_ROW, np.int64)
    w = np.zeros(cap_total, np.float32)
    g[pos] = dsts[tok]
    s[pos] = srcs[tok]
    w[pos] = ws[tok]
    return g, s, w, total_chunks


def _split_core(src, dst, w, s):
    """Partition a core's edges into pass-1 (per-dst-block round-0, capped)
    and mini (everything else).  Returns pass-1 per-block arrays + mini
    edge lists."""
    sel = (src >= NHALF) == bool(s)
    srcs = src[sel] - s * NHALF
    dsts = dst[sel]
    ws = w[sel]

    blk = dsts >> 7                       # dst block, 0..390
    o = np.lexsort((srcs, blk))
    sb, ss2 = blk[o], srcs[o]
    first = np.r_[True, (sb[1:] != sb[:-1]) | (ss2[1:] != ss2[:-1])]
    gs = np.flatnonzero(first)
    occ_sorted = np.arange(len(o)) - np.repeat(gs, np.diff(np.r_[gs, len(o)]))
    occ = np.empty(len(o), np.int64)
    occ[o] = occ_sorted

    r0_idx = np.flatnonzero(occ == 0)
    blk_r0 = blk[r0_idx]
    o2 = np.argsort(blk_r0, kind="stable")
    counts = np.bincount(blk_r0, minlength=NBLK)
    starts = np.concatenate([[0], np.cumsum(counts[:-1])])
    k_in_blk = np.arange(len(o2)) - np.repeat(starts, counts)
    keep = k_in_blk < CAP1

    p1 = r0_idx[o2[keep]]                 # edge ids, block-major order
    p1_blk = blk_r0[o2[keep]]
    p1_k = k_in_blk[keep]

    mini_mask = occ > 0
    mini_mask[r0_idx[o2[~keep]]] = True

    return {
        "p1_blk": p1_blk, "p1_k": p1_k,
        "p1_dl": (dsts[p1] & 127).astype(np.int64),
        "p1_src": srcs[p1], "p1_w": ws[p1],
        "m_src": srcs[mini_mask], "m_dst": dsts[mini_mask],
        "m_w": ws[mini_mask],
    }


def _prep_core(sp, ch_mini):
    """Build the per-core input map from _split_core output."""
    out = {}
    # one-hot matrices, w folded in
    oh = np.zeros((NBLK, 128, CAP1), np.float32)
    oh[sp["p1_blk"], sp["p1_dl"], sp["p1_k"]] = sp["p1_w"]
    out["oh"] = np.ascontiguousarray(
        oh.astype(BF16).reshape(QD, 4, 128, CAP1).transpose(0, 2, 1, 3)
    )
    del oh
    # scatter indices
    sfull = np.full((NBLK, CAP1), PAD_ROW, np.int64)
    sfull[sp["p1_blk"], sp["p1_k"]] = sp["p1_src"]
    sx = _wrap16(sfull.reshape(-1), NBLK, CAP1)      # [NBLK, 128, 72]
    out["sx"] = np.ascontiguousarray(
        sx.reshape(NBLK // 8, 8, 128, CAP1 // 16).transpose(0, 2, 1, 3)
    )
    # mini pass
    cap = ch_mini * C
    g_all = np.zeros((2, cap), np.int64)
    s_all = np.full((2, cap), PAD_ROW, np.int64)
    w_all = np.zeros((2, cap), np.float32)
    for phase in range(2):
        pm = (sp["m_dst"] >= NHALF) == bool(phase)
        g, sarr, warr, nch = _round_pack(
            sp["m_src"][pm], sp["m_dst"][pm] - phase * NHALF, sp["m_w"][pm], C)
        assert nch <= ch_mini, f"mini overflow: {nch} > {ch_mini}"
        g_all[phase, :nch * C] = g
        s_all[phase, :nch * C] = sarr
        w_all[phase, :nch * C] = warr
    out["gidx"] = np.stack([_wrap16(g_all[p], ch_mini, C) for p in range(2)])
    out["sidx"] = np.stack([_wrap16(s_all[p], ch_mini, C) for p in range(2)])
    out["wl"] = np.ascontiguousarray(
        w_all.reshape(2, ch_mini, C // 128, 128).transpose(0, 1, 3, 2)
    )
    return out


def _mini_chunks(sp):
    worst = 1
    for phase in range(2):
        pm = (sp["m_dst"] >= NHALF) == bool(phase)
        ss = sp["m_src"][pm]
        if ss.size == 0:
            continue
        cnts = np.bincount(ss)
        mx = int(cnts.max())
        rounds = np.array([(cnts > r).sum() for r in range(mx)])
        worst = max(worst, int(np.sum(-(-rounds // C))))
    return worst


def kernel(**inputs):
    H = np.ascontiguousarray(np.asarray(inputs["H"], np.float32))
    w = np.asarray(inputs["edge_w"], np.float32)
    src = np.asarray(inputs["edge_src"], np.int64)
    dst = np.asarray(inputs["edge_dst"], np.int64)

    splits = []
    ch_mini = 1
    for core in range(8):
        b, s = core // 2, core % 2
        sp = _split_core(src[b], dst[b], w[b], s)
        ch_mini = max(ch_mini, _mini_chunks(sp))
        splits.append(sp)

    nc = _get_compiled(ch_mini)

    # blocked bf16 H: hb[p, j, f] = H[b, j*128+p, f]
    hb_all = np.zeros((B, 128, NBLK, HS), BF16)
    for b in range(B):
        hpad = np.zeros((NBLK * 128, HS), np.float32)
        hpad[:N] = H[b]
        hb_all[b] = hpad.reshape(NBLK, 128, HS).transpose(1, 0, 2).astype(BF16)

    in_maps = []
    for core in range(8):
        b = core // 2
        m = _prep_core(splits[core], ch_mini)
        m["h"] = H[b]
        m["hb"] = hb_all[b]
        in_maps.append(m)

    trace = bool(int(os.environ.get("GNN_TRACE", "0")))
    res = run_bass_kernel_spmd(nc, in_maps, list(range(8)), trace=trace)
    LAST_RESULT["exec_time_ns"] = res.exec_time_ns
    LAST_RESULT["res"] = res

    out = np.empty((B, N, HS), np.float32)
    for core in range(8):
        b, s = core // 2, core % 2
        acc = res.results[core]["acc"].sum(axis=0)  # [2, 128, NGRP, HS]
        rows = acc.transpose(2, 0, 1, 3).reshape(-1, HS)[:NHALF]
        out[b, s * NHALF:(s + 1) * NHALF] = rows
    return out


# revision 9
# speedup vs baseline: 1.0048x; 1.0048x over previous
"""Neighbor aggregation (gnn message passing) Bass kernel for Trainium2.

out[b, i] = sum_{e: src[e]==i} w[e] * H[b, dst[e]]   (per batch b)

8 NeuronCores: core = 2*b + s handles batch b, src-half s (output rows
[s*25000, (s+1)*25000)).

v2 architecture (replaces the SWDGE dma_gather per-edge path, which was
Pool-engine descriptor-generation bound at ~9 ns/token):

  - Gather is done on the Tensor engine: edges are grouped host-side by
    128-row dst-block; per block a one-hot matrix OnehotT [128 dst x 1024
    edge-slots] (bf16, with w folded in) is streamed from HBM and used as
    the stationary matmul operand against the H block [128 x 64] (bf16,
    resident in SBUF), producing messages w*H[dst] token-major in PSUM.
  - Scatter stays on SWDGE dma_scatter_add (CCE f32, parity-split SBUF
    accumulators, one 1024-token call per dst-block; pad tails use the
    junk row PAD_ROW).
  - RMW-hazard safety: each per-block call only contains round-0 edges
    (first occurrence of each src within the block) -> all srcs within a
    call are distinct.  Remaining edges (~2%: in-block src duplicates and
    over-cap overflow) go through a small baseline-style SWDGE
    gather+scatter pass with round-packing.

Hardware constraints (probed previously):
  - SWDGE calls limited to 1024-1152 tokens (descriptor ring packets).
  - dma_scatter_add loses RMW updates when the same dst row repeats in
    close proximity within a call; hence distinct-src calls + PAD_ROW junk
    row (>= 25000) for padding tokens.
"""

import os
import sys

sys.path.insert(0, "/opt/trn_rl_repo")

import numpy as np
import ml_dtypes

import concourse.bacc as bacc
import concourse.mybir as mybir
import concourse.tile as tile
from concourse.bass_utils import run_bass_kernel_spmd

BF16 = ml_dtypes.bfloat16

B, N, E, HS = 4, 50000, 800000, 64
NHALF = N // 2                  # 25000
C = 1024                        # tokens per mini-pass chunk
NGRP = 98                       # accumulator covers idx < 25088
PAD_ROW = 25080                 # junk accumulator row for padding tokens

NBLK = 392                      # dst blocks of 128 rows (covers 50176)
G1 = 8                          # message groups per block
CAP1 = G1 * 128                 # 1024 edge slots per block
QD = NBLK // 4                  # onehot DMA batches (4 blocks each)

LAST_RESULT = {}


def build(nc, ch_mini):
    f32 = mybir.dt.float32
    bf16 = mybir.dt.bfloat16
    i16 = mybir.dt.int16
    i32 = mybir.dt.int32

    h_d = nc.dram_tensor("h", [N, HS], f32, kind="ExternalInput")
    hb_d = nc.dram_tensor("hb", [128, NBLK, HS], bf16, kind="ExternalInput")
    oh_d = nc.dram_tensor("oh", [QD, 128, 4, CAP1], bf16, kind="ExternalInput")
    sx_d = nc.dram_tensor(
        "sx", [NBLK // 8, 128, 8, CAP1 // 16], i16, kind="ExternalInput"
    )
    gidx_d = nc.dram_tensor(
        "gidx", [2, ch_mini, 128, C // 16], i16, kind="ExternalInput"
    )
    sidx_d = nc.dram_tensor(
        "sidx", [2, ch_mini, 128, C // 16], i16, kind="ExternalInput"
    )
    wl_d = nc.dram_tensor(
        "wl", [2, ch_mini, 128, C // 128], f32, kind="ExternalInput"
    )
    acc_d = nc.dram_tensor("acc", [2, 2, 128, NGRP, HS], f32, kind="ExternalOutput")

    with tile.TileContext(nc) as tc:
        with tc.tile_pool(name="accp", bufs=1) as accp, \
             tc.tile_pool(name="hp", bufs=1) as hp, \
             tc.tile_pool(name="ohp", bufs=2) as ohp, \
             tc.tile_pool(name="sxp", bufs=2) as sxp, \
             tc.tile_pool(name="msgp", bufs=4) as msgp, \
             tc.tile_pool(name="pp", bufs=6, space="PSUM") as pp, \
             tc.tile_pool(name="mp", bufs=1) as mp, \
             tc.tile_pool(name="wp", bufs=1) as wp:
            accs = []
            for pr in range(2):
                a0 = accp.tile([128, NGRP, HS], f32, tag=f"acc{pr}0")
                a1 = accp.tile([128, NGRP, HS], f32, tag=f"acc{pr}1")
                nc.vector.memset(a0[:], 0.0)
                nc.vector.memset(a1[:], 0.0)
                accs.append((a0, a1))

            hb_t = hp.tile([128, NBLK, HS], bf16, tag="hb")
            nc.sync.dma_start(hb_t[:], hb_d[:])

            sx_t = None
            for q in range(QD):
                oh_t = ohp.tile([128, 4, CAP1], bf16, tag="oh")
                nc.sync.dma_start(oh_t[:], oh_d[q])
                if q % 2 == 0:
                    sx_t = sxp.tile([128, 8, CAP1 // 16], i16, tag="sx")
                    nc.sync.dma_start(sx_t[:], sx_d[q // 2])
                for bq in range(4):
                    j = q * 4 + bq
                    ps = pp.tile([128, G1, HS], f32, tag="ps")
                    for g in range(G1):
                        nc.tensor.matmul(
                            ps[:, g, :],
                            lhsT=oh_t[:, bq, g * 128:(g + 1) * 128],
                            rhs=hb_t[:, j, :],
                            start=True,
                            stop=True,
                        )
                    msg_t = msgp.tile([128, G1, HS], f32, tag="msg")
                    nc.vector.tensor_copy(msg_t[:], ps[:])
                    a0, a1 = accs[j % 2]
                    nc.gpsimd.dma_scatter_add(
                        out_ap=a0[:],
                        in_ap=msg_t[:],
                        idxs_ap=sx_t[:, bq + 4 * (q % 2), :],
                        num_idxs=CAP1,
                        num_idxs_reg=CAP1,
                        elem_size=HS,
                        sbuf_tokens_per_rank=128,
                        parity_reg=0,
                        out_ap_other=a1[:],
                    )

            # mini pass: src-duplicate and overflow edges via SWDGE gather.
            # All gathers of a phase issue before its scatters so the Pool
            # FIFO never head-blocks on the gather->multiply->scatter chain.
            for phase in range(2):
                h_slice = h_d[:][phase * NHALF:(phase + 1) * NHALF, :]
                msgs_list, si_list = [], []
                for k in range(ch_mini):
                    gi = wp.tile([128, C // 16], i16, tag=f"gi{k}")
                    si = wp.tile([128, C // 16], i16, tag=f"si{k}")
                    wt = wp.tile([128, C // 128], f32, tag=f"wt{k}")
                    nc.sync.dma_start(gi[:], gidx_d[phase, k])
                    nc.sync.dma_start(si[:], sidx_d[phase, k])
                    nc.sync.dma_start(wt[:], wl_d[phase, k])

                    msgs = wp.tile([128, C // 128, HS], f32, tag=f"ms{k}")
                    nc.gpsimd.dma_gather(
                        out_ap=msgs[:],
                        in_ap=h_slice,
                        idxs_ap=gi[:],
                        num_idxs=C,
                        num_idxs_reg=C,
                        elem_size=HS,
                    )
                    nc.vector.tensor_tensor(
                        out=msgs[:],
                        in0=msgs[:],
                        in1=wt[:].unsqueeze(2).broadcast_to([128, C // 128, HS]),
                        op=mybir.AluOpType.mult,
                    )
                    msgs_list.append(msgs)
                    si_list.append(si)
                for k in range(ch_mini):
                    a0, a1 = accs[(k + phase * ch_mini) % 2]
                    nc.gpsimd.dma_scatter_add(
                        out_ap=a0[:],
                        in_ap=msgs_list[k][:],
                        idxs_ap=si_list[k][:],
                        num_idxs=C,
                        num_idxs_reg=C,
                        elem_size=HS,
                        sbuf_tokens_per_rank=128,
                        parity_reg=0,
                        out_ap_other=a1[:],
                    )

            for pr in range(2):
                nc.sync.dma_start(acc_d[pr, 0], accs[pr][0][:])
                nc.sync.dma_start(acc_d[pr, 1], accs[pr][1][:])
    return nc


_COMPILED = {}


def _get_compiled(ch_mini):
    if ch_mini not in _COMPILED:
        nc = bacc.Bacc("TRN2", target_bir_lowering=False, debug=False)
        build(nc, ch_mini)
        nc.compile()
        _COMPILED[ch_mini] = nc
    return _COMPILED[ch_mini]


def _wrap16(idx, ch, c):
    a = idx.reshape(ch, c // 16, 16).transpose(0, 2, 1).astype(np.int16)
    return np.ascontiguousarray(np.tile(a, (1, 8, 1)))


def _round_pack(srcs, dsts, ws, cap):
    """Order tokens so equal src never share a chunk: tokens get a within-src
    rank (round); each round starts at a fresh chunk boundary."""
    order = np.argsort(srcs, kind="stable")
    ss = srcs[order]
    n = ss.shape[0]
    if n == 0:
        return (np.zeros(cap, np.int64), np.full(cap, PAD_ROW, np.int64),
                np.zeros(cap, np.float32), 1)
    first = np.r_[True, ss[1:] != ss[:-1]]
    gstart = np.flatnonzero(first)
    rank = np.arange(n) - np.repeat(gstart, np.diff(np.r_[gstart, n]))
    order2 = np.argsort(rank, kind="stable")
    rank_s = rank[order2]
    tok = order[order2]
    nr = np.bincount(rank_s)
    chunks_per_round = -(-nr // cap)
    starts = np.concatenate([[0], np.cumsum(chunks_per_round[:-1] * cap)])
    total_chunks = int(chunks_per_round.sum())
    pos = starts[rank_s] + (np.arange(n) - np.repeat(
        np.concatenate([[0], np.cumsum(nr[:-1])]), nr))
    cap_total = total_chunks * cap
    g = np.zeros(cap_total, np.int64)
    s = np.full(cap_total, PAD_ROW, np.int64)
    w = np.zeros(cap_total, np.float32)
    g[pos] = dsts[tok]
    s[pos] = srcs[tok]
    w[pos] = ws[tok]
    return g, s, w, total_chunks


def _split_core(src, dst, w, s):
    """Partition a core's edges into pass-1 (per-dst-block round-0, capped)
    and mini (everything else).  Returns pass-1 per-block arrays + mini
    edge lists."""
    sel = (src >= NHALF) == bool(s)
    srcs = src[sel] - s * NHALF
    dsts = dst[sel]
    ws = w[sel]

    blk = dsts >> 7                       # dst block, 0..390
    o = np.lexsort((srcs, blk))
    sb, ss2 = blk[o], srcs[o]
    first = np.r_[True, (sb[1:] != sb[:-1]) | (ss2[1:] != ss2[:-1])]
    gs = np.flatnonzero(first)
    occ_sorted = np.arange(len(o)) - np.repeat(gs, np.diff(np.r_[gs, len(o)]))
    occ = np.empty(len(o), np.int64)
    occ[o] = occ_sorted

    r0_idx = np.flatnonzero(occ == 0)
    blk_r0 = blk[r0_idx]
    o2 = np.argsort(blk_r0, kind="stable")
    counts = np.bincount(blk_r0, minlength=NBLK)
    starts = np.concatenate([[0], np.cumsum(counts[:-1])])
    k_in_blk = np.arange(len(o2)) - np.repeat(starts, counts)
    keep = k_in_blk < CAP1

    p1 = r0_idx[o2[keep]]                 # edge ids, block-major order
    p1_blk = blk_r0[o2[keep]]
    p1_k = k_in_blk[keep]

    mini_mask = occ > 0
    mini_mask[r0_idx[o2[~keep]]] = True

    return {
        "p1_blk": p1_blk, "p1_k": p1_k,
        "p1_dl": (dsts[p1] & 127).astype(np.int64),
        "p1_src": srcs[p1], "p1_w": ws[p1],
        "m_src": srcs[mini_mask], "m_dst": dsts[mini_mask],
        "m_w": ws[mini_mask],
    }


def _prep_core(sp, ch_mini):
    """Build the per-core input map from _split_core output."""
    out = {}
    # one-hot matrices, w folded in
    oh = np.zeros((NBLK, 128, CAP1), np.float32)
    oh[sp["p1_blk"], sp["p1_dl"], sp["p1_k"]] = sp["p1_w"]
    out["oh"] = np.ascontiguousarray(
        oh.astype(BF16).reshape(QD, 4, 128, CAP1).transpose(0, 2, 1, 3)
    )
    del oh
    # scatter indices
    sfull = np.full((NBLK, CAP1), PAD_ROW, np.int64)
    sfull[sp["p1_blk"], sp["p1_k"]] = sp["p1_src"]
    sx = _wrap16(sfull.reshape(-1), NBLK, CAP1)      # [NBLK, 128, 72]
    out["sx"] = np.ascontiguousarray(
        sx.reshape(NBLK // 8, 8, 128, CAP1 // 16).transpose(0, 2, 1, 3)
    )
    # mini pass
    cap = ch_mini * C
    g_all = np.zeros((2, cap), np.int64)
    s_all = np.full((2, cap), PAD_ROW, np.int64)
    w_all = np.zeros((2, cap), np.float32)
    for phase in range(2):
        pm = (sp["m_dst"] >= NHALF) == bool(phase)
        g, sarr, warr, nch = _round_pack(
            sp["m_src"][pm], sp["m_dst"][pm] - phase * NHALF, sp["m_w"][pm], C)
        assert nch <= ch_mini, f"mini overflow: {nch} > {ch_mini}"
        g_all[phase, :nch * C] = g
        s_all[phase, :nch * C] = sarr
        w_all[phase, :nch * C] = warr
    out["gidx"] = np.stack([_wrap16(g_all[p], ch_mini, C) for p in range(2)])
    out["sidx"] = np.stack([_wrap16(s_all[p], ch_mini, C) for p in range(2)])
    out["wl"] = np.ascontiguousarray(
        w_all.reshape(2, ch_mini, C // 128, 128).transpose(0, 1, 3, 2)
    )
    return out


def _mini_chunks(sp):
    worst = 1
    for phase in range(2):
        pm = (sp["m_dst"] >= NHALF) == bool(phase)
        ss = sp["m_src"][pm]
        if ss.size == 0:
            continue
        cnts = np.bincount(ss)
        mx = int(cnts.max())
        rounds = np.array([(cnts > r).sum() for r in range(mx)])
        worst = max(worst, int(np.sum(-(-rounds // C))))
    return worst


def kernel(**inputs):
    H = np.ascontiguousarray(np.asarray(inputs["H"], np.float32))
    w = np.asarray(inputs["edge_w"], np.float32)
    src = np.asarray(inputs["edge_src"], np.int64)
    dst = np.asarray(inputs["edge_dst"], np.int64)

    splits = []
    ch_mini = 1
    for core in range(8):
        b, s = core // 2, core % 2
        sp = _split_core(src[b], dst[b], w[b], s)
        ch_mini = max(ch_mini, _mini_chunks(sp))
        splits.append(sp)

    nc = _get_compiled(ch_mini)

    # blocked bf16 H: hb[p, j, f] = H[b, j*128+p, f]
    hb_all = np.zeros((B, 128, NBLK, HS), BF16)
    for b in range(B):
        hpad = np.zeros((NBLK * 128, HS), np.float32)
        hpad[:N] = H[b]
        hb_all[b] = hpad.reshape(NBLK, 128, HS).transpose(1, 0, 2).astype(BF16)

    in_maps = []
    for core in range(8):
        b = core // 2
        m = _prep_core(splits[core], ch_mini)
        m["h"] = H[b]
        m["hb"] = hb_all[b]
        in_maps.append(m)

    trace = bool(int(os.environ.get("GNN_TRACE", "0")))
    res = run_bass_kernel_spmd(nc, in_maps, list(range(8)), trace=trace)
    LAST_RESULT["exec_time_ns"] = res.exec_time_ns
    LAST_RESULT["res"] = res

    out = np.empty((B, N, HS), np.float32)
    for core in range(8):
        b, s = core // 2, core % 2
        acc = res.results[core]["acc"].sum(axis=0)  # [2, 128, NGRP, HS]
        rows = acc.transpose(2, 0, 1, 3).reshape(-1, HS)[:NHALF]
        out[b, s * NHALF:(s + 1) * NHALF] = rows
    return out
